# revision 12
# baseline (speedup 1.0000x reference)
"""Bahdanau additive attention on 8 Trainium2 NeuronCores (Bass/Tile).

Math (per batch element b):
  sq = query @ W2 + b2                 [TQ, U]
  sv = values @ W1 + b1                [TK, U]
  logits[q,k] = sum_u V[u] * tanh(sq[q,u] + sv[k,u])   (+ V_b, which cancels
                                                        in the softmax)
  A = softmax(logits, axis=k)          [TQ, TK]
  ctx = A @ values                     [TQ, DV]

Sharding: data-parallel over the batch axis (B=8) across 8 cores; weights
replicated. No collectives.

Kernel strategy per core:
  * PE transposes query/values (identity matmul), projects to sq_T[u,q] and
    sv_T[u,k] (SBUF, biases folded into sv_T).
  * Broadcast add sq_T + sv_T[:,k]: split between ScalarE (fused bias-tanh)
    and VectorE (tensor_scalar add, then batched big-FD tanh on ScalarE),
    per 16-k block, emitting bf16 tanh tiles T_k[u, q].
  * u-contraction with V: stationary = V replicated across all 128 PE
    columns; per k one N=256 matmul whose 32-row output slice (column-group
    tile position) packs 4 k's per PSUM tile.  Logits come out [k, q]-
    transposed; strided DMA extracts 4 rows per tile.
  * exp in transposed layout; softmax denominators via ones-column matmul
    (S[q] = sum_k E_T[k,q]); normalization fused into PSUM->SBUF copies.
  * context = E_T.T @ values directly (E_T is already the needed lhsT).
"""

import numpy as np
from contextlib import ExitStack

import concourse.bass as bass
import concourse.tile as tile
import concourse.mybir as mybir
from concourse import bacc
from concourse.bass_utils import run_bass_kernel_spmd
from concourse.masks import make_identity

AF = mybir.ActivationFunctionType
F32 = mybir.dt.float32
BF16 = mybir.dt.bfloat16

B, TQ, TK, DQ, DV, U = 8, 256, 256, 512, 512, 256
P = 128
DT = DQ // P   # 4 d-tiles
UC = U // P    # 2 u-tiles
QC = TQ // P   # 2 q-tiles
KC = TK // P   # 2 k-tiles

KB = 16                 # k-block size
NBLK = TK // KB         # 16 blocks
# every Nth block uses the ScalarE fused bias-tanh path (rest: DVE add +
# batched big-FD tanh)
ACT_BLOCK_EVERY = 4


def _build():
    nc = bacc.Bacc(
        "TRN2",
        target_bir_lowering=False,
        debug=False,
        enable_asserts=False,
        num_devices=B,
    )
    q_dram = nc.dram_tensor("query", [TQ, DQ], F32, kind="ExternalInput").ap()
    v_dram = nc.dram_tensor("values", [TK, DV], F32, kind="ExternalInput").ap()
    w1_dram = nc.dram_tensor("W1_w", [DV, U], F32, kind="ExternalInput").ap()
    b1_dram = nc.dram_tensor("W1_b", [U], F32, kind="ExternalInput").ap()
    w2_dram = nc.dram_tensor("W2_w", [DQ, U], F32, kind="ExternalInput").ap()
    b2_dram = nc.dram_tensor("W2_b", [U], F32, kind="ExternalInput").ap()
    vw_dram = nc.dram_tensor("V_w", [U, 1], F32, kind="ExternalInput").ap()
    ctx_dram = nc.dram_tensor("context", [TQ, DV], F32, kind="ExternalOutput").ap()
    att_dram = nc.dram_tensor("attw", [TQ, TK], F32, kind="ExternalOutput").ap()

    with tile.TileContext(nc) as tc, ExitStack() as ctx:
        singles = ctx.enter_context(tc.tile_pool(name="singles", bufs=1))
        tk_pool = ctx.enter_context(tc.tile_pool(name="tkp", bufs=4))
        pre_pool = ctx.enter_context(tc.tile_pool(name="prep", bufs=3))
        out_pool = ctx.enter_context(tc.tile_pool(name="outs", bufs=2))
        stg_pool = ctx.enter_context(tc.tile_pool(name="stg", bufs=6))
        ps_ex = ctx.enter_context(tc.tile_pool(name="ps_ex", bufs=4, space="PSUM"))
        ps_s = ctx.enter_context(tc.tile_pool(name="ps_s", bufs=1, space="PSUM"))
        ps_big = ctx.enter_context(tc.tile_pool(name="ps_big", bufs=1, space="PSUM"))

        # ---- Phase 0: load everything --------------------------------
        ident = singles.tile([P, P], F32)
        make_identity(nc, ident)

        q_nat = singles.tile([P, QC, DQ], F32)   # q_nat[p, t, d] = query[t*128+p, d]
        nc.sync.dma_start(q_nat, q_dram.rearrange("(t p) d -> p t d", p=P))
        v_nat = singles.tile([P, KC, DV], F32)
        nc.sync.dma_start(v_nat, v_dram.rearrange("(t p) d -> p t d", p=P))
        w1_sb = singles.tile([P, DT, U], F32)    # w1_sb[p, t, u] = W1[t*128+p, u]
        nc.sync.dma_start(w1_sb, w1_dram.rearrange("(t p) u -> p t u", p=P))
        w2_sb = singles.tile([P, DT, U], F32)
        nc.sync.dma_start(w2_sb, w2_dram.rearrange("(t p) u -> p t u", p=P))
        v_col = singles.tile([P, UC], F32)       # v_col[p, c] = V_w[c*128+p, 0]
        nc.sync.dma_start(v_col, vw_dram.rearrange("(c p) o -> p (c o)", p=P))
        b1_sb = singles.tile([P, UC], F32)
        nc.sync.dma_start(b1_sb, b1_dram.rearrange("(c p) -> p c", p=P))
        b2_sb = singles.tile([P, UC], F32)
        nc.sync.dma_start(b2_sb, b2_dram.rearrange("(c p) -> p c", p=P))
        bsum = singles.tile([P, UC], F32)        # W1_b + W2_b per u
        nc.vector.tensor_add(bsum, b1_sb, b2_sb)

        # V replicated across 128 PE columns, bf16, per u-half
        vrep = singles.tile([P, UC, P], BF16)
        for uc in range(UC):
            nc.vector.tensor_copy(
                vrep[:, uc, :], v_col[:, uc:uc + 1].to_broadcast((P, P)))
        ones_col = singles.tile([P, 1], F32)
        nc.vector.memset(ones_col, 1.0)

        # ---- Phase 1: transpose query/values to [d, *] ---------------
        qT = singles.tile([P, DT, TQ], F32)      # qT[p, dc, q] = query[q, dc*128+p]
        vT = singles.tile([P, DT, TK], F32)
        for (nat, dst, tmax) in ((q_nat, qT, QC), (v_nat, vT, KC)):
            for t in range(tmax):
                for dc in range(DT):
                    ps = ps_ex.tile([P, 256], F32, tag="ex", name="tr")
                    nc.tensor.transpose(
                        ps[:, :P], nat[:, t, dc * P:(dc + 1) * P], ident)
                    if dst is qT:
                        nc.scalar.copy(dst[:, dc, t * P:(t + 1) * P], ps[:, :P])
                    else:
                        nc.vector.tensor_copy(dst[:, dc, t * P:(t + 1) * P],
                                              ps[:, :P])

        # ---- Phase 2: projections ------------------------------------
        sqT_sb = singles.tile([P, UC, TQ], F32)  # sq_T[u, q] (no bias)
        svT_sb = singles.tile([P, UC, TK], F32)  # sv_T[u, k] + (b1+b2)
        for uc in range(UC):
            sq_ps = ps_ex.tile([P, 256], F32, tag="ex", name="sq_ps")
            for dt in range(DT):
                nc.tensor.matmul(
                    sq_ps,
                    lhsT=w2_sb[:, dt, uc * P:(uc + 1) * P],
                    rhs=qT[:, dt, :],
                    start=(dt == 0),
                    stop=(dt == DT - 1),
                )
            nc.vector.tensor_copy(sqT_sb[:, uc, :], sq_ps)
            sv_ps = ps_ex.tile([P, 256], F32, tag="ex", name="sv_ps")
            for dt in range(DT):
                nc.tensor.matmul(
                    sv_ps,
                    lhsT=w1_sb[:, dt, uc * P:(uc + 1) * P],
                    rhs=vT[:, dt, :],
                    start=(dt == 0),
                    stop=(dt == DT - 1),
                )
            # sv_T + (W1_b + W2_b) folded in during the PSUM->SBUF copy
            nc.scalar.activation(
                svT_sb[:, uc, :], sv_ps, AF.Identity, bias=bsum[:, uc:uc + 1]
            )

        # ---- Phase 3: tanh + V-contraction over u --------------------
        # logits_T[k, q] partials per u-half, assembled in SBUF
        partials = singles.tile([P, UC, KC, TQ], F32)
        for b in range(NBLK):
            mode = "act" if b % 8 == 0 else "dve"
            tkb = tk_pool.tile([P, UC, KB, TQ], BF16, tag="tkb")
            if mode == "act":
                for kl in range(KB):
                    k = KB * b + kl
                    for uc in range(UC):
                        nc.scalar.activation(
                            tkb[:, uc, kl, :], sqT_sb[:, uc, :], AF.Tanh,
                            bias=svT_sb[:, uc, k:k + 1],
                        )
            else:
                adder = nc.vector if mode == "dve" else nc.gpsimd
                for h in range(2):
                    tpre = pre_pool.tile([P, UC, KB // 2, TQ], F32, tag="tpre")
                    for kl in range(KB // 2):
                        k = KB * b + (KB // 2) * h + kl
                        for uc in range(UC):
                            adder.tensor_scalar_add(
                                tpre[:, uc, kl, :], sqT_sb[:, uc, :],
                                svT_sb[:, uc, k:k + 1],
                            )
                    for uc in range(UC):
                        nc.scalar.activation(
                            tkb[:, uc, (KB // 2) * h:(KB // 2) * (h + 1), :],
                            tpre[:, uc], AF.Tanh,
                        )
            # V-contraction: per uc, replicated-V stationary; 4 k's per tile
            # (rows 32j carry k = 4t+j, replicated 32x). PSUM -> SBUF full
            # copy (alternating ScalarE/VectorE), then a partition-strided
            # SBUF->SBUF DMA packs the 4 logit rows into `partials`.
            for uc in range(UC):
                for t in range(KB // 8):
                    ps = ps_ex.tile([P, 512], F32, tag="ex", name="lg")
                    for j in range(4):
                        # group j computes k = k0+j (free half 0) and
                        # k = k0+4+j (free half 1) so each extraction DMA
                        # writes a contiguous partition range
                        kl = 8 * t + j
                        nc.tensor.matmul(
                            ps[32 * j:32 * (j + 1), :],
                            lhsT=vrep[:, uc, 32 * j:32 * (j + 1)],
                            rhs=tkb[:, uc, kl:kl + 5:4, :],
                            start=True, stop=True,
                            tile_position=(0, 32 * j),
                        )
                    stg = stg_pool.tile([P, 512], F32, tag="stg")
                    nc.scalar.copy(stg, ps)
                    k0 = KB * b + 8 * t
                    for i in range(2):
                        nc.sync.dma_start(
                            partials[(k0 % P) + 4 * i:(k0 % P) + 4 * i + 4,
                                     uc, k0 // P, :],
                            stg[0:P:32, i * 256:(i + 1) * 256],
                        )

        # ---- Phase 4: combine partials, exp --------------------------
        logT = singles.tile([P, KC, TQ], F32)    # logits_T[k, kc, q]
        nc.vector.tensor_add(logT, partials[:, 0], partials[:, 1])
        eT = singles.tile([P, KC, TQ], F32)      # exp(logits_T)
        nc.scalar.activation(eT, logT, AF.Exp)

        # S[q] = sum_k E_T[k, q] via ones-column matmul; then 1/S
        s_ps = ps_s.tile([P, QC], F32)
        for qc in range(QC):
            for kc in range(KC):
                nc.tensor.matmul(
                    s_ps[:, qc:qc + 1],
                    lhsT=eT[:, kc, qc * P:(qc + 1) * P],
                    rhs=ones_col,
                    start=(kc == 0),
                    stop=(kc == KC - 1),
                )
        rsum = out_pool.tile([P, QC], F32, tag="rsum")
        nc.vector.reciprocal(rsum, s_ps)

        # ---- Phase 5: attention weights out (transpose + normalize) --
        for qc in range(QC):
            aw = out_pool.tile([P, TK], F32, tag=f"aw{qc}")
            for kc in range(KC):
                ps = ps_ex.tile([P, 256], F32, tag="ex", name="atr")
                nc.tensor.transpose(
                    ps[:, :P], eT[:, kc, qc * P:(qc + 1) * P], ident)
                nc.vector.tensor_scalar_mul(
                    aw[:, kc * P:(kc + 1) * P], ps[:, :P], rsum[:, qc:qc + 1])
            nc.sync.dma_start(att_dram[qc * P:(qc + 1) * P, :], aw)

        # ---- Phase 6: context = (E_T.T @ values) / S -----------------
        for qc in range(QC):
            ctx_ps = ps_big.tile([P, DV], F32, tag="ctx")
            for kc in range(KC):
                nc.tensor.matmul(
                    ctx_ps,
                    lhsT=eT[:, kc, qc * P:(qc + 1) * P],
                    rhs=v_nat[:, kc, :],
                    start=(kc == 0),
                    stop=(kc == KC - 1),
                )
            ctx_sb = out_pool.tile([P, DV], F32, tag=f"ctx{qc}")
            nc.vector.tensor_scalar_mul(ctx_sb, ctx_ps, rsum[:, qc:qc + 1])
            nc.sync.dma_start(ctx_dram[qc * P:(qc + 1) * P, :], ctx_sb)

    nc.compile()
    return nc


_NC_CACHE = {}


def _get_nc():
    if "nc" not in _NC_CACHE:
        _NC_CACHE["nc"] = _build()
    return _NC_CACHE["nc"]


def _run(in_maps, **kwargs):
    nc = _get_nc()
    return run_bass_kernel_spmd(nc, in_maps, core_ids=list(range(B)), **kwargs)


def make_in_maps(query, values, W1_w, W1_b, W2_w, W2_b, V_w, V_b=None):
    def f32(x):
        return np.ascontiguousarray(np.asarray(x, dtype=np.float32))

    query, values = f32(query), f32(values)
    shared = {
        "W1_w": f32(W1_w), "W1_b": f32(W1_b),
        "W2_w": f32(W2_w), "W2_b": f32(W2_b),
        "V_w": f32(V_w),
    }
    return [dict(query=query[b], values=values[b], **shared) for b in range(B)]


def kernel(query, values, W1_w, W1_b, W2_w, W2_b, V_w, V_b, **run_kwargs):
    in_maps = make_in_maps(query, values, W1_w, W1_b, W2_w, W2_b, V_w)
    res = _run(in_maps, **run_kwargs)
    context = np.stack([res.results[b]["context"] for b in range(B)])
    attw = np.stack([res.results[b]["attw"] for b in range(B)])
    kernel.last_results = res
    return context, attw


# revision 13
# speedup vs baseline: 1.0660x; 1.0660x over previous
"""Bahdanau additive attention on 8 Trainium2 NeuronCores (Bass/Tile).

Math (per batch element b):
  sq = query @ W2 + b2                 [TQ, U]
  sv = values @ W1 + b1                [TK, U]
  logits[q,k] = sum_u V[u] * tanh(sq[q,u] + sv[k,u])   (+ V_b, which cancels
                                                        in the softmax)
  A = softmax(logits, axis=k)          [TQ, TK]
  ctx = A @ values                     [TQ, DV]

Sharding: data-parallel over the batch axis (B=8) across 8 cores; weights
replicated. No collectives.

Kernel strategy per core:
  * PE transposes query/values (identity matmul), projects to sq_T[u,q] and
    sv_T[u,k] (SBUF, biases folded into sv_T).
  * Broadcast add sq_T + sv_T[:,k]: split between ScalarE (fused bias-tanh)
    and VectorE (tensor_scalar add, then batched big-FD tanh on ScalarE),
    per 16-k block, emitting bf16 tanh tiles T_k[u, q].
  * u-contraction with V: stationary = V replicated across all 128 PE
    columns; per k one N=256 matmul whose 32-row output slice (column-group
    tile position) packs 4 k's per PSUM tile.  Logits come out [k, q]-
    transposed; strided DMA extracts 4 rows per tile.
  * exp in transposed layout; softmax denominators via ones-column matmul
    (S[q] = sum_k E_T[k,q]); normalization fused into PSUM->SBUF copies.
  * context = E_T.T @ values directly (E_T is already the needed lhsT).
"""

import numpy as np
from contextlib import ExitStack

import concourse.bass as bass
import concourse.tile as tile
import concourse.mybir as mybir
from concourse import bacc
from concourse.bass_utils import run_bass_kernel_spmd
from concourse.masks import make_identity

AF = mybir.ActivationFunctionType
F32 = mybir.dt.float32
BF16 = mybir.dt.bfloat16

B, TQ, TK, DQ, DV, U = 8, 256, 256, 512, 512, 256
P = 128
DT = DQ // P   # 4 d-tiles
UC = U // P    # 2 u-tiles
QC = TQ // P   # 2 q-tiles
KC = TK // P   # 2 k-tiles

KB = 16                 # k-block size
NBLK = TK // KB         # 16 blocks
# every Nth block uses the ScalarE fused bias-tanh path (rest: DVE add +
# batched big-FD tanh)
ACT_BLOCK_EVERY = 4


def _build():
    nc = bacc.Bacc(
        "TRN2",
        target_bir_lowering=False,
        debug=False,
        enable_asserts=False,
        num_devices=B,
    )
    q_dram = nc.dram_tensor("query", [TQ, DQ], F32, kind="ExternalInput").ap()
    v_dram = nc.dram_tensor("values", [TK, DV], F32, kind="ExternalInput").ap()
    w1_dram = nc.dram_tensor("W1_w", [DV, U], F32, kind="ExternalInput").ap()
    b1_dram = nc.dram_tensor("W1_b", [U], F32, kind="ExternalInput").ap()
    w2_dram = nc.dram_tensor("W2_w", [DQ, U], F32, kind="ExternalInput").ap()
    b2_dram = nc.dram_tensor("W2_b", [U], F32, kind="ExternalInput").ap()
    vw_dram = nc.dram_tensor("V_w", [U, 1], F32, kind="ExternalInput").ap()
    ctx_dram = nc.dram_tensor("context", [TQ, DV], F32, kind="ExternalOutput").ap()
    att_dram = nc.dram_tensor("attw", [TQ, TK], F32, kind="ExternalOutput").ap()

    with tile.TileContext(nc) as tc, ExitStack() as ctx:
        singles = ctx.enter_context(tc.tile_pool(name="singles", bufs=1))
        tk_pool = ctx.enter_context(tc.tile_pool(name="tkp", bufs=4))
        pre_pool = ctx.enter_context(tc.tile_pool(name="prep", bufs=2))
        out_pool = ctx.enter_context(tc.tile_pool(name="outs", bufs=2))
        stg_pool = ctx.enter_context(tc.tile_pool(name="stg", bufs=6))
        ps_ex = ctx.enter_context(tc.tile_pool(name="ps_ex", bufs=4, space="PSUM"))
        ps_s = ctx.enter_context(tc.tile_pool(name="ps_s", bufs=1, space="PSUM"))
        ps_big = ctx.enter_context(tc.tile_pool(name="ps_big", bufs=1, space="PSUM"))

        # ---- Phase 0: load everything --------------------------------
        ident = singles.tile([P, P], F32)
        make_identity(nc, ident)

        q_nat = singles.tile([P, QC, DQ], F32)   # q_nat[p, t, d] = query[t*128+p, d]
        nc.sync.dma_start(q_nat, q_dram.rearrange("(t p) d -> p t d", p=P))
        v_nat = singles.tile([P, KC, DV], F32)
        nc.sync.dma_start(v_nat, v_dram.rearrange("(t p) d -> p t d", p=P))
        w1_sb = singles.tile([P, DT, U], F32)    # w1_sb[p, t, u] = W1[t*128+p, u]
        nc.sync.dma_start(w1_sb, w1_dram.rearrange("(t p) u -> p t u", p=P))
        w2_sb = singles.tile([P, DT, U], F32)
        nc.sync.dma_start(w2_sb, w2_dram.rearrange("(t p) u -> p t u", p=P))
        v_col = singles.tile([P, UC], F32)       # v_col[p, c] = V_w[c*128+p, 0]
        nc.sync.dma_start(v_col, vw_dram.rearrange("(c p) o -> p (c o)", p=P))
        b1_sb = singles.tile([P, UC], F32)
        nc.sync.dma_start(b1_sb, b1_dram.rearrange("(c p) -> p c", p=P))
        b2_sb = singles.tile([P, UC], F32)
        nc.sync.dma_start(b2_sb, b2_dram.rearrange("(c p) -> p c", p=P))
        bsum = singles.tile([P, UC], F32)        # W1_b + W2_b per u
        nc.vector.tensor_add(bsum, b1_sb, b2_sb)

        # V replicated across 128 PE columns, bf16, per u-half
        vrep = singles.tile([P, UC, P], BF16)
        for uc in range(UC):
            nc.vector.tensor_copy(
                vrep[:, uc, :], v_col[:, uc:uc + 1].to_broadcast((P, P)))
        ones_col = singles.tile([P, 1], F32)
        nc.vector.memset(ones_col, 1.0)

        # ---- Phase 1: transpose query/values to [d, *] ---------------
        qT = singles.tile([P, DT, TQ], F32)      # qT[p, dc, q] = query[q, dc*128+p]
        vT = singles.tile([P, DT, TK], F32)
        for (nat, dst, tmax) in ((q_nat, qT, QC), (v_nat, vT, KC)):
            for t in range(tmax):
                for dc in range(DT):
                    ps = ps_ex.tile([P, 256], F32, tag="ex", name="tr")
                    nc.tensor.transpose(
                        ps[:, :P], nat[:, t, dc * P:(dc + 1) * P], ident)
                    if dst is qT:
                        nc.scalar.copy(dst[:, dc, t * P:(t + 1) * P], ps[:, :P])
                    else:
                        nc.vector.tensor_copy(dst[:, dc, t * P:(t + 1) * P],
                                              ps[:, :P])

        # ---- Phase 2: projections ------------------------------------
        sqT_sb = singles.tile([P, UC, TQ], F32)  # sq_T[u, q] (no bias)
        svT_sb = singles.tile([P, UC, TK], F32)  # sv_T[u, k] + (b1+b2)
        for uc in range(UC):
            sq_ps = ps_ex.tile([P, 256], F32, tag="ex", name="sq_ps")
            for dt in range(DT):
                nc.tensor.matmul(
                    sq_ps,
                    lhsT=w2_sb[:, dt, uc * P:(uc + 1) * P],
                    rhs=qT[:, dt, :],
                    start=(dt == 0),
                    stop=(dt == DT - 1),
                )
            nc.vector.tensor_copy(sqT_sb[:, uc, :], sq_ps)
            sv_ps = ps_ex.tile([P, 256], F32, tag="ex", name="sv_ps")
            for dt in range(DT):
                nc.tensor.matmul(
                    sv_ps,
                    lhsT=w1_sb[:, dt, uc * P:(uc + 1) * P],
                    rhs=vT[:, dt, :],
                    start=(dt == 0),
                    stop=(dt == DT - 1),
                )
            # sv_T + (W1_b + W2_b) folded in during the PSUM->SBUF copy
            nc.scalar.activation(
                svT_sb[:, uc, :], sv_ps, AF.Identity, bias=bsum[:, uc:uc + 1]
            )

        # ---- Phase 3: tanh + V-contraction over u --------------------
        # logits_T[k, q] partials per u-half, assembled in SBUF
        partials = singles.tile([P, UC, KC, TQ], F32)
        for b in range(NBLK):
            mode = "act" if b == 0 else "dve"
            tkb = tk_pool.tile([P, UC, KB, TQ], BF16, tag="tkb")
            if mode == "act":
                for kl in range(KB):
                    k = KB * b + kl
                    for uc in range(UC):
                        nc.scalar.activation(
                            tkb[:, uc, kl, :], sqT_sb[:, uc, :], AF.Tanh,
                            bias=svT_sb[:, uc, k:k + 1],
                        )
            else:
                # one TT add per (uc, 8-k group): in0 = sq_T broadcast along
                # k, in1 = sv_T k-slice broadcast along q; FD = 8*256
                tpre = pre_pool.tile([P, UC, KB, TQ], F32, tag="tpre")
                G = 8
                for uc in range(UC):
                    for g in range(KB // G):
                        k = KB * b + G * g
                        nc.vector.tensor_tensor(
                            tpre[:, uc, G * g:G * (g + 1), :],
                            sqT_sb[:, uc, None, :].to_broadcast((P, G, TQ)),
                            svT_sb[:, uc, k:k + G, None].to_broadcast(
                                (P, G, TQ)),
                            mybir.AluOpType.add,
                        )
                # single big-FD tanh over the whole block (both u-halves)
                nc.scalar.activation(tkb, tpre, AF.Tanh)
            # V-contraction: per uc, replicated-V stationary; 4 k's per tile
            # (rows 32j carry k = 4t+j, replicated 32x). PSUM -> SBUF full
            # copy (alternating ScalarE/VectorE), then a partition-strided
            # SBUF->SBUF DMA packs the 4 logit rows into `partials`.
            for uc in range(UC):
                for t in range(KB // 8):
                    ps = ps_ex.tile([P, 512], F32, tag="ex", name="lg")
                    for j in range(4):
                        # group j computes k = k0+j (free half 0) and
                        # k = k0+4+j (free half 1) so each extraction DMA
                        # writes a contiguous partition range
                        kl = 8 * t + j
                        nc.tensor.matmul(
                            ps[32 * j:32 * (j + 1), :],
                            lhsT=vrep[:, uc, 32 * j:32 * (j + 1)],
                            rhs=tkb[:, uc, kl:kl + 5:4, :],
                            start=True, stop=True,
                            tile_position=(0, 32 * j),
                        )
                    stg = stg_pool.tile([P, 512], F32, tag="stg")
                    nc.scalar.copy(stg, ps)
                    k0 = KB * b + 8 * t
                    for i in range(2):
                        nc.sync.dma_start(
                            partials[(k0 % P) + 4 * i:(k0 % P) + 4 * i + 4,
                                     uc, k0 // P, :],
                            stg[0:P:32, i * 256:(i + 1) * 256],
                        )

        # ---- Phase 4: combine partials, exp --------------------------
        logT = singles.tile([P, KC, TQ], F32)    # logits_T[k, kc, q]
        nc.vector.tensor_add(logT, partials[:, 0], partials[:, 1])
        eT = singles.tile([P, KC, TQ], F32)      # exp(logits_T)
        nc.scalar.activation(eT, logT, AF.Exp)

        # S[q] = sum_k E_T[k, q] via ones-column matmul; then 1/S
        s_ps = ps_s.tile([P, QC], F32)
        for qc in range(QC):
            for kc in range(KC):
                nc.tensor.matmul(
                    s_ps[:, qc:qc + 1],
                    lhsT=eT[:, kc, qc * P:(qc + 1) * P],
                    rhs=ones_col,
                    start=(kc == 0),
                    stop=(kc == KC - 1),
                )
        rsum = out_pool.tile([P, QC], F32, tag="rsum")
        nc.vector.reciprocal(rsum, s_ps)

        # ---- Phase 5: attention weights out (transpose + normalize) --
        for qc in range(QC):
            aw = out_pool.tile([P, TK], F32, tag=f"aw{qc}")
            for kc in range(KC):
                ps = ps_ex.tile([P, 256], F32, tag="ex", name="atr")
                nc.tensor.transpose(
                    ps[:, :P], eT[:, kc, qc * P:(qc + 1) * P], ident)
                nc.vector.tensor_scalar_mul(
                    aw[:, kc * P:(kc + 1) * P], ps[:, :P], rsum[:, qc:qc + 1])
            nc.sync.dma_start(att_dram[qc * P:(qc + 1) * P, :], aw)

        # ---- Phase 6: context = (E_T.T @ values) / S -----------------
        for qc in range(QC):
            ctx_ps = ps_big.tile([P, DV], F32, tag="ctx")
            for kc in range(KC):
                nc.tensor.matmul(
                    ctx_ps,
                    lhsT=eT[:, kc, qc * P:(qc + 1) * P],
                    rhs=v_nat[:, kc, :],
                    start=(kc == 0),
                    stop=(kc == KC - 1),
                )
            ctx_sb = out_pool.tile([P, DV], F32, tag=f"ctx{qc}")
            nc.vector.tensor_scalar_mul(ctx_sb, ctx_ps, rsum[:, qc:qc + 1])
            nc.sync.dma_start(ctx_dram[qc * P:(qc + 1) * P, :], ctx_sb)

    nc.compile()
    return nc


_NC_CACHE = {}


def _get_nc():
    if "nc" not in _NC_CACHE:
        _NC_CACHE["nc"] = _build()
    return _NC_CACHE["nc"]


def _run(in_maps, **kwargs):
    nc = _get_nc()
    return run_bass_kernel_spmd(nc, in_maps, core_ids=list(range(B)), **kwargs)


def make_in_maps(query, values, W1_w, W1_b, W2_w, W2_b, V_w, V_b=None):
    def f32(x):
        return np.ascontiguousarray(np.asarray(x, dtype=np.float32))

    query, values = f32(query), f32(values)
    shared = {
        "W1_w": f32(W1_w), "W1_b": f32(W1_b),
        "W2_w": f32(W2_w), "W2_b": f32(W2_b),
        "V_w": f32(V_w),
    }
    return [dict(query=query[b], values=values[b], **shared) for b in range(B)]


def kernel(query, values, W1_w, W1_b, W2_w, W2_b, V_w, V_b, **run_kwargs):
    in_maps = make_in_maps(query, values, W1_w, W1_b, W2_w, W2_b, V_w)
    res = _run(in_maps, **run_kwargs)
    context = np.stack([res.results[b]["context"] for b in range(B)])
    attw = np.stack([res.results[b]["attw"] for b in range(B)])
    kernel.last_results = res
    return context, attw
